# revision 13
# baseline (speedup 1.0000x reference)
"""Trainium2 Bass kernel for nn_DecoderSmoothedMaxPoolingLoss.

Loss (see reference):
  neg  = -log(1 - X)                                    (B,T,K)
  loss = sum_{b, t<len_b, k} neg
         - sum_{b, i in [0,Lw_b), k=tgt_b} neg[b, tau_s_b + i, k]
         + sum_b -log( max_j  clip(conv_same(win_b * valid_b, filt), EPS, 1) * valid_b )
  where tau_s = max(0, w_end + 40 - 60), tau_e = min(tau_s + 60, len),
  Lw = tau_e - tau_s, win_b[i] = X[b, tau_s_b + i, tgt_b].

Sharding: pure data parallel over batch — 8 batches per core on 8 cores.
Each core computes its partial scalar loss on device; host sums the 8
partials (the "all-reduce").

Per core (8 local batches = 12.8 MB), viewed as (128, 25000):
partition p <-> batch b=p//16, t in [250*(p%16), 250*(p%16)+250), k inner.

  bulk:     3 large HWDGE (nc.sync) chunk loads with 40/30/30 KB
            per-partition descriptors (large descriptors amortize the
            per-descriptor HBM latency that capped the old 6.4 KB-desc
            load at ~11 GB/s/engine).
  neg term: 10 ACT Ln blocks of (128, 2500) (25 t-rows each), each
            writing its per-partition block sum via accum_out — the
            reduction rides the activation pass, so DVE never touches
            the bulk data.  Host-built block mask (-1 = block fully
            inside t<len, 0 otherwise) dots with the block sums.
  boundary: the one partial 25-row block per batch (t=len_b not a
            multiple of 25) is re-gathered (8 x 25 rows), Ln'd and
            masked elementwise — small correction path.
  windows:  indirect gather of 60 rows/batch -> one-hot select of
            k=tgt -> exclusion term, and conv via two small matmuls ->
            clip/mask/max -> positive term (same as before).
  final:    all partial columns -> one matmul with ones -> scalar out.
"""

import numpy as np

import concourse.bass as bass
import concourse.tile as tile
from concourse import bacc
from concourse import mybir
from concourse import bass_utils
from concourse.bass import IndirectOffsetOnAxis

AF = mybir.ActivationFunctionType
ALU = mybir.AluOpType
AX = mybir.AxisListType
FP = mybir.dt.float32
I32 = mybir.dt.int32

B, T, K = 64, 4000, 100
WIN, OFFSET_D, TRUNC, SIGMA = 60, 40, 21, 9
EPS = 1e-8
NCORES = 8
BLOC = B // NCORES          # 8 batches per core
P = 128                     # partitions
FTOT = BLOC * T * K // P    # 25000 free elems per partition
TR = 250                    # t-rows per partition
BR = 25                     # t-rows per ACT block
NBLK = TR // BR             # 10 ACT blocks
FB = BR * K                 # 2500 free elems per block
CHUNK_BLKS = (4, 4, 2)      # DMA chunks in units of ACT blocks

WA = WIN * K + WIN + BLOC + FB      # auxA (8, 8568): ohrep | valid8 | I8 | mb2
WBF = WIN + NBLK                    # auxB (128, 70): conv M (rows<60) | Mblk


def _filt_np():
    half = TRUNC // 2
    x = np.arange(-half, half + 1, dtype=np.float32)
    g = np.exp(-0.5 * (x / SIGMA) ** 2).astype(np.float32)
    g = g / g.sum()
    f = np.zeros(WIN, np.float32)
    c = WIN // 2
    f[c - half:c + half + 1] = g
    return f


def _conv_matrix():
    # smoothed[j] = sum_i win[i] * filt[i - j + pl], pl = (WIN-1)//2
    f = _filt_np()
    pl = (WIN - 1) // 2
    idx = np.arange(WIN)
    u = idx[:, None] - idx[None, :] + pl          # (i, j)
    M = np.where((u >= 0) & (u < WIN), f[np.clip(u, 0, WIN - 1)], 0.0)
    return M.astype(np.float32)


_NC_CACHE = None


def _build_program():
    global _NC_CACHE
    if _NC_CACHE is not None:
        return _NC_CACHE

    nc = bacc.Bacc("TRN2", debug=False)
    Xs = nc.dram_tensor("Xs", [BLOC, T, K], FP, kind="ExternalInput").ap()
    auxA = nc.dram_tensor("auxA", [BLOC, WA], FP, kind="ExternalInput").ap()
    auxB = nc.dram_tensor("auxB", [P, WBF], FP, kind="ExternalInput").ap()
    gidx = nc.dram_tensor("gidx", [BLOC, 2], I32, kind="ExternalInput").ap()
    outd = nc.dram_tensor("out", [1, 1], FP, kind="ExternalOutput").ap()

    with tile.TileContext(nc) as tc:
        with tc.tile_pool(name="xin", bufs=1) as xin_pool, \
             tc.tile_pool(name="small", bufs=1) as small, \
             tc.tile_pool(name="psum", bufs=1, space="PSUM") as psum:

            # ---------- bulk chunk 0 first (SWDGE ring head) ----------
            Xp = Xs.rearrange("b t k -> (b t k)").rearrange(
                "(p f) -> p f", p=P)                     # (128, 25000)
            xt = []
            base = 0
            for ci, nb in enumerate(CHUNK_BLKS):
                xt.append(xin_pool.tile([P, nb * FB], FP, tag=f"xb{ci}",
                                        name=f"xb{ci}"))
            nc.gpsimd.dma_start(out=xt[0][:], in_=Xp[:, 0:CHUNK_BLKS[0] * FB])

            # ---------- aux loads (HWDGE sync queue: land in ~2 us,
            # independent of the SWDGE bulk ring) ----------
            gidx_sb = small.tile([BLOC, 2], I32)
            nc.sync.dma_start(out=gidx_sb[:], in_=gidx)
            auxA_sb = small.tile([BLOC, WA], FP)
            nc.sync.dma_start(out=auxA_sb[:], in_=auxA)
            auxB_sb = small.tile([P, WBF], FP)
            nc.sync.dma_start(out=auxB_sb[:], in_=auxB)

            ohrep_sl = auxA_sb[0:BLOC, 0:WIN * K]
            valid_sl = auxA_sb[0:BLOC, WIN * K:WIN * K + WIN]
            I8_sl = auxA_sb[0:BLOC, WIN * K + WIN:WIN * K + WIN + BLOC]
            mb2_sl = auxA_sb[0:BLOC, WIN * K + WIN + BLOC:WA]
            M_sl = auxB_sb[0:WIN, 0:WIN]
            Mblk_sl = auxB_sb[0:P, WIN:WBF]

            # ---------- indirect gathers (SWDGE) ----------
            Xrows = Xs.rearrange("b t k -> (b t) k")
            Wp = small.tile([BLOC, WIN * K], FP)
            nc.gpsimd.indirect_dma_start(
                out=Wp[:], out_offset=None, in_=Xrows,
                in_offset=IndirectOffsetOnAxis(ap=gidx_sb[:, 0:1], axis=0))
            Wb = small.tile([BLOC, FB], FP)
            nc.gpsimd.indirect_dma_start(
                out=Wb[:], out_offset=None, in_=Xrows,
                in_offset=IndirectOffsetOnAxis(ap=gidx_sb[:, 1:2], axis=0))

            # ---------- bulk chunks 1,2 behind the gathers ----------
            base = CHUNK_BLKS[0] * FB
            for ci in range(1, len(CHUNK_BLKS)):
                f = CHUNK_BLKS[ci] * FB
                nc.gpsimd.dma_start(out=xt[ci][:], in_=Xp[:, base:base + f])
                base += f

            # ---------- result columns ----------
            Cfin = small.tile([P, 4], FP)
            nc.vector.memset(Cfin[:], 0.0)
            scr8 = small.tile([BLOC, FB], FP)

            # ---------- boundary-block correction (ACT early) ----------
            nc.scalar.activation(out=Wb[:], in_=Wb[:], func=AF.Ln,
                                 bias=1.0, scale=-1.0)
            nc.vector.tensor_tensor(out=scr8[:], in0=Wb[:], in1=mb2_sl,
                                    op=ALU.mult)
            nc.vector.tensor_reduce(out=Cfin[0:BLOC, 1:2], in_=scr8[:],
                                    axis=AX.X, op=ALU.add)

            # ---------- window path ----------
            nc.vector.tensor_tensor(out=Wp[:], in0=Wp[:], in1=ohrep_sl,
                                    op=ALU.mult)
            win_raw = small.tile([BLOC, WIN], FP)
            nc.vector.tensor_reduce(
                out=win_raw[:],
                in_=Wp[:].rearrange("b (i k) -> b i k", k=K),
                axis=AX.X, op=ALU.add)
            winv = small.tile([BLOC, WIN], FP)
            nc.vector.tensor_tensor(out=winv[:], in0=win_raw[:],
                                    in1=valid_sl, op=ALU.mult)
            # exclusion: + sum_i valid * ln(1 - win_raw)
            lnw = small.tile([BLOC, WIN], FP)
            nc.scalar.activation(out=lnw[:], in_=win_raw[:], func=AF.Ln,
                                 bias=1.0, scale=-1.0)
            lnwv = small.tile([BLOC, WIN], FP)
            nc.vector.tensor_tensor(out=lnwv[:], in0=lnw[:], in1=valid_sl,
                                    op=ALU.mult)
            nc.vector.tensor_reduce(out=Cfin[0:BLOC, 2:3], in_=lnwv[:],
                                    axis=AX.X, op=ALU.add)
            # smoothed = win_v @ M (transpose first via identity)
            wvt_ps = psum.tile([WIN, BLOC], FP)
            nc.tensor.matmul(out=wvt_ps[:], lhsT=winv[:], rhs=I8_sl,
                             start=True, stop=True)
            wvt = small.tile([WIN, BLOC], FP)
            nc.vector.tensor_copy(out=wvt[:], in_=wvt_ps[:])
            sm_ps = psum.tile([BLOC, WIN], FP)
            nc.tensor.matmul(out=sm_ps[:], lhsT=wvt[:], rhs=M_sl,
                             start=True, stop=True)
            smc = small.tile([BLOC, WIN], FP)
            nc.vector.tensor_scalar(out=smc[:], in0=sm_ps[:],
                                    scalar1=EPS, scalar2=1.0,
                                    op0=ALU.max, op1=ALU.min)
            smv = small.tile([BLOC, WIN], FP)
            nc.vector.tensor_tensor(out=smv[:], in0=smc[:], in1=valid_sl,
                                    op=ALU.mult)
            mx = small.tile([BLOC, 1], FP)
            nc.vector.tensor_reduce(out=mx[:], in_=smv[:], axis=AX.X,
                                    op=ALU.max)
            lnmx = small.tile([BLOC, 1], FP)
            nc.scalar.activation(out=lnmx[:], in_=mx[:], func=AF.Ln)
            nc.vector.tensor_scalar_mul(Cfin[0:BLOC, 3:4], lnmx[:], -1.0)

            # ---------- neg term: 10 ACT Ln blocks w/ accum ----------
            AC = small.tile([P, NBLK], FP)
            g = 0
            for c, nb in enumerate(CHUNK_BLKS):
                for l in range(nb):
                    sl = xt[c][:, l * FB:(l + 1) * FB]
                    nc.scalar.activation(
                        out=sl, in_=sl, func=AF.Ln, bias=1.0, scale=-1.0,
                        accum_out=AC[:, g:g + 1])
                    g += 1

            # dot block sums with block mask
            scrA = small.tile([P, NBLK], FP)
            nc.vector.tensor_tensor(out=scrA[:], in0=AC[:], in1=Mblk_sl,
                                    op=ALU.mult)
            nc.vector.tensor_reduce(out=Cfin[0:P, 0:1], in_=scrA[:],
                                    axis=AX.X, op=ALU.add)

            # ---------- final partition reduce ----------
            ones = small.tile([P, 1], FP)
            nc.vector.memset(ones[:], 1.0)
            tot_ps = psum.tile([1, 4], FP)
            nc.tensor.matmul(out=tot_ps[:], lhsT=ones[:], rhs=Cfin[:],
                             start=True, stop=True)
            tot = small.tile([1, 1], FP)
            nc.vector.tensor_reduce(out=tot[:], in_=tot_ps[:], axis=AX.X,
                                    op=ALU.add)
            nc.gpsimd.dma_start(out=outd, in_=tot[:])

    nc.compile()
    _NC_CACHE = nc
    return nc


def _make_in_maps(X, lengths, tgt, w_end):
    X = np.ascontiguousarray(np.asarray(X, dtype=np.float32))
    lengths = np.asarray(lengths, dtype=np.int64)
    tgt = np.asarray(tgt, dtype=np.int64)
    w_end = np.asarray(w_end, dtype=np.int64)

    tau_s = np.maximum(0, w_end + OFFSET_D - WIN)
    tau_e = np.minimum(tau_s + WIN, lengths)
    Lw = tau_e - tau_s

    Mmat = _conv_matrix()
    I8 = np.eye(BLOC, dtype=np.float32)

    in_maps = []
    for cr in range(NCORES):
        bs = slice(cr * BLOC, (cr + 1) * BLOC)
        ls, ts, lw, tg = lengths[bs], tau_s[bs], Lw[bs], tgt[bs]

        oh = np.zeros((BLOC, K), np.float32)
        oh[np.arange(BLOC), tg] = 1.0
        ohrep = np.broadcast_to(oh[:, None, :], (BLOC, WIN, K)) \
            .reshape(BLOC, WIN * K)
        valid8 = (np.arange(WIN)[None, :] < lw[:, None]).astype(np.float32)

        # boundary block: rows [25*floor(len/25), len) when len % 25 != 0
        bstart = (ls // BR) * BR
        rstar = ls - bstart                       # 0..24
        rowmask = (np.arange(BR)[None, :] < rstar[:, None])
        mb2 = -np.repeat(rowmask, K, axis=1).astype(np.float32)  # (8, 2500)
        auxA = np.concatenate([ohrep, valid8, I8, mb2], axis=1)

        # Mblk[p, g] = -1 iff block [250*(p%16)+25g, +25) fully < len_b
        pidx = np.arange(P)
        qq = (pidx % (P // BLOC)) * TR            # 250*(p%16)
        bb = pidx // (P // BLOC)                  # batch of partition
        gidx_blk = np.arange(NBLK)
        blk_end = qq[:, None] + BR * (gidx_blk[None, :] + 1)
        Mblk = -(blk_end <= ls[bb][:, None]).astype(np.float32)  # (128, 10)

        Mpad = np.zeros((P, WIN), np.float32)
        Mpad[0:WIN] = Mmat
        auxB = np.concatenate([Mpad, Mblk], axis=1)              # (128, 70)

        off_win = (np.arange(BLOC) * T + ts).astype(np.int32)
        off_bnd = np.where(rstar > 0, np.arange(BLOC) * T + bstart, 0) \
            .astype(np.int32)
        gidx_arr = np.stack([off_win, off_bnd], axis=1)          # (8, 2) i32

        in_maps.append({
            "Xs": np.ascontiguousarray(X[bs]),
            "auxA": np.ascontiguousarray(auxA),
            "auxB": np.ascontiguousarray(auxB),
            "gidx": np.ascontiguousarray(gidx_arr),
        })
    return in_maps


def kernel(X, lengths, tgt, w_end):
    nc = _build_program()
    in_maps = _make_in_maps(X, lengths, tgt, w_end)
    res = bass_utils.run_bass_kernel_spmd(
        nc, in_maps, core_ids=list(range(NCORES)))
    total = np.float32(0.0)
    for c in range(NCORES):
        total += np.float32(res.results[c]["out"][0, 0])
    return np.array(total, dtype=np.float32)
